# revision 73
# baseline (speedup 1.0000x reference)
"""Trainium2 Bass kernel for nn_BERTNet_75256416961146.

Pipeline per sentence (B=64 sentences, sharded 8/core over 8 NeuronCores):
  1. segment-mean of h[b] [512,768] over sorted seg_ids -> means [256,768]
     (computed transposed as means^T [768,256] = h^T @ A, A = one-hot of seg_ids)
  2. P_stack [512,640] = [means @ W1_top ; means @ W1_bot] (row r is the
     left/right MLP projection of token r mod 256), scaled by 1/cnt per token,
     evicted to fp8e4 q-pair tiles
  3. pre^T [640,1024] = P_stack^T-gather via fp8e4 DoubleRow matmuls against
     the one-hot G^T (exact in fp8); tanh(+b1) in one [128,1024] ACT op per m
  4. logits^T [4,1024] = W2^T @ hid^T (bf16); exp(+b2); 8 PE transposes to
     config-partition layout; softmax normalize (tiny DVE ops); DMA out.

means/P_stack/logits matmuls in bf16, gather in fp8e4 DoubleRow (2 k-tiles
per instruction), all with fp32 PSUM accumulation. One DMA per sentence for
h ([p, q, d] layout), single conf DMA, consolidated weight staging.

HW-measured choices (trn2, axon): fp8 DoubleRow gather -40us; bf16 A-build
(2x DVE) large win; f32r matmuls slower than bf16+cast; gpsimd tensor_scalar
and ACT Copy casts catastrophically slow (+100us); bf16 softmax slower.
fp8 on P_stack/logits inputs exceeds the 2e-2 error budget (each stage alone
is ~0.013-0.018 rel; they RSS-combine).
"""

import os
import numpy as np
from contextlib import ExitStack

os.environ.setdefault("MYCRO_LOCAL_CACHE", "1")

import concourse.bass as bass
import concourse.tile as tile
from concourse import mybir
from concourse import library_config
from concourse.bass_utils import run_bass_kernel_spmd

# ---- problem shapes (hardcoded per contest rules) ----
B, S, T, C = 64, 512, 256, 1024
D, MLP, CLS = 768, 600, 4
NCORES = 8
BPC = B // NCORES          # sentences per core
P = 128
SCH = S // P               # 4 subtoken chunks
DCH = D // P               # 6 hidden chunks
TCH = T // P               # 2 token chunks
MLPP = 640                 # MLP padded to 5*128
MCH = MLPP // P            # 5 mlp chunks
NH = C // 512              # 2 config halves for psum tiling
CJ = C // P                # 8 config blocks of 128

f32 = mybir.dt.float32
bf16 = mybir.dt.bfloat16
i32 = mybir.dt.int32
Alu = mybir.AluOpType
Act = mybir.ActivationFunctionType

# Timing aid: build the module with the whole per-sentence body repeated
# REPEATS times (same data, same outputs) so kernel time can be separated
# from host dispatch overhead. Grading path always uses REPEATS=1.
REPEATS = 1

# ---- tuning knobs (swept via TimelineSim, validated on HW) ----
def _knob(name, default):
    return os.environ.get(f"K_{name}", default)


def _flag(name, default):
    return os.environ.get(f"K_{name}", "1" if default else "0") == "1"


MEANS_EVICT_ENGINE = _knob("MEANS_EVICT", "vector")  # PSUM->SBUF means eviction
PSTACK_EVICT_ENGINE = _knob("PSTACK_EVICT", "vector")  # scale+cast eviction engine
# h f32->bf16 cast engines, one per q-chunk (when MEANS_F32R off)
HB_CAST_ENGINES = _knob("HB_CAST", "vector,vector,vector,vector").split(",")
PSTACK_BUFS = int(_knob("PSTACK_BUFS", "1"))  # psum bufs for P_stack tiles (2 banks each)
PRE_BUFS = int(_knob("PRE_BUFS", "2"))        # psum bufs for pre tiles (1 bank each)
PSUM_PLAN = _knob("PSUM_PLAN", "X")  # "A": separate pools; "X": cnt/pre/logits/expT share one
PIPELINE_HEAD = True         # emit next sentence's loads/casts/A mid-way through current
HEAD_AFTER = _knob("HEAD_AFTER", "cnt")  # emit next head after cnt (155 vs 168us A/B)
# counts on DVE (compare+reduce, needs segbc broadcast) vs 8 tiny bf16 PE
# matmuls; PE variant measured slower back-to-back (208 vs 190us)
CNT_VIA_DVE = _flag("CNT_VIA_DVE", True)
SOFTMAX_ENGINE = _knob("SOFTMAX_ENGINE", "pe")  # "pe": 8 PE transposes; "dve": DVE 32x32 transpose
SOFTMAX_BF16 = _flag("SOFTMAX_BF16", False)  # bf16 softmax measured slower on HW
MEANS_F32R = _flag("MEANS_F32R", False)  # f32r matmuls measured slower than bf16+cast on HW
PSTACK_FP8 = _flag("PSTACK_FP8", False)  # fp8 means+W1 alone costs rel-err 0.024: off
GATHER_FP8 = _flag("GATHER_FP8", True)   # pre-gather matmul in fp8e4 DoubleRow (~0.014 err)
LOGITS_FP8 = _flag("LOGITS_FP8", False)  # no PE win (DoubleRow M=4 invalid) but adds err
f32r = mybir.dt.float32r
fp8 = mybir.dt.float8e4
DR = mybir.MatmulPerfMode.DoubleRow


def _cast(nc, out, in_, engine="vector"):
    # dtype-converting copy. On DVE/gpsimd use tensor_scalar (lowers to
    # TensorScalarPtr, which supports multiple sync waits; DVE TensorCopy's
    # TR struct does not). On the scalar engine use an Activation copy.
    if engine == "scalar":
        nc.scalar.activation(out, in_, Act.Copy)
    elif engine == "gpsimd":
        nc.gpsimd.tensor_scalar(out, in_, 1.0, None, Alu.mult)
    else:
        nc.vector.tensor_scalar(out, in_, 1.0, None, Alu.mult)


def _body(ctx, tc, nc, h_d, seg_d, conf_d, w1_d, b1_d, w2_d, b2_d, out_d,
          iota_row_d, iota_col_d, ident_d):
    const = ctx.enter_context(tc.tile_pool(name="const", bufs=1))
    wstage = ctx.enter_context(tc.tile_pool(name="wstage", bufs=1))

    # ---------- constants ----------
    iota_row = const.tile([P, T], f32)          # 0..255 along free, all partitions
    nc.sync.dma_start(iota_row[:], iota_row_d)
    iota_row_bf = const.tile([P, T], bf16)      # bf16 copy: A build runs 2x on DVE
    _cast(nc, iota_row_bf[:], iota_row[:])
    iota_col = const.tile([P, 1], f32)          # partition index
    nc.sync.dma_start(iota_col[:], iota_col_d)
    iota_col2 = const.tile([P, 1], f32)         # partition index + 128
    nc.vector.tensor_scalar(iota_col2[:], iota_col[:], 128.0, None, Alu.add)
    if SOFTMAX_ENGINE == "pe":
        ident4 = const.tile([CLS, CLS], f32)
        nc.sync.dma_start(ident4[:], ident_d)
    ones_f = const.tile([P, 1], f32r if MEANS_F32R else bf16)
    # memset can't write f32r; synthesize 1.0 = iota*0 + 1 on DVE instead
    nc.vector.tensor_scalar(ones_f[:], iota_col[:], 0.0, 1.0, Alu.mult, Alu.add)

    # ---------- weights (emitted lazily, after sentence-0's input DMAs, so
    # the first means matmuls are not starved behind 3.6MB of W1 DMA) ----------
    wt = {}

    def _load_weights():
        # W1 staged as [d-pair, j, half*MLPP + m] for DoubleRow k-tiles (j in
        # {0,1} selects the d-chunk within the pair). One DMA for all of W1.
        w1dt = fp8 if PSTACK_FP8 else bf16
        w1s = const.tile([P, DCH // 2, 2, 2 * MLPP], w1dt)
        for half in range(2):  # zero only the MLP..MLPP pad columns
            nc.vector.memset(w1s[:, :, :, half * MLPP + MLP: (half + 1) * MLPP], 0.0)
        for half in range(2):
            st = wstage.tile([P, DCH, MLP], f32, name="st", tag="st")
            nc.sync.dma_start(st[:], w1_d[half * D:(half + 1) * D, :].rearrange(
                "(g p) m -> p g m", p=P))
            for k in range(DCH):
                _cast(nc, w1s[:, k // 2, k % 2, half * MLPP: half * MLPP + MLP],
                      st[:, k, :])

        w2dt = fp8 if LOGITS_FP8 else bf16
        w2s = const.tile([P, MCH, CLS], w2dt)
        nc.vector.memset(w2s[:, MCH - 1, :], 0.0)
        st2 = wstage.tile([P, 4, CLS], f32, name="st2", tag="st2")
        nc.sync.dma_start(st2[:], w2_d[0:512, :].rearrange("(m p) c -> p m c", p=P))
        st2b = wstage.tile([P, CLS], f32, name="st2b", tag="st2b")
        nc.sync.dma_start(st2b[:MLP - 512], w2_d[512:MLP, :])
        for m in range(4):
            _cast(nc, w2s[:, m, :], st2[:, m, :])
        _cast(nc, w2s[:MLP - 512, MCH - 1, :], st2b[:MLP - 512])

        b1s = const.tile([P, MCH], f32)
        nc.vector.memset(b1s[:], 0.0)
        nc.sync.dma_start(b1s[:, 0:4], b1_d[0:512].rearrange("(m p) -> p m", p=P))
        nc.sync.dma_start(b1s[:MLP - 512, 4:5], b1_d[512:MLP].unsqueeze(-1))

        b2c = const.tile([CLS, 1], f32)
        nc.sync.dma_start(b2c[:], b2_d.unsqueeze(-1))
        wt.update(w1s=w1s, w2s=w2s, b1s=b1s, b2c=b2c)

    # ---------- per-sentence pools ----------
    hf_pool = ctx.enter_context(tc.tile_pool(name="hf", bufs=3))
    hb_pool = None if MEANS_F32R else ctx.enter_context(tc.tile_pool(name="hb", bufs=12))
    seg_pool = ctx.enter_context(tc.tile_pool(name="segp", bufs=6))
    a_pool = ctx.enter_context(tc.tile_pool(name="ap", bufs=8))
    means_pool = ctx.enter_context(tc.tile_pool(name="meansp", bufs=8))
    recip_pool = ctx.enter_context(tc.tile_pool(name="recipp", bufs=8))
    pstack_pool = ctx.enter_context(tc.tile_pool(name="pstackp", bufs=8 if GATHER_FP8 else 4))
    conf_pool = ctx.enter_context(tc.tile_pool(name="confp", bufs=2))
    idxbc_pool = ctx.enter_context(tc.tile_pool(name="idxbcp", bufs=2))
    gt_pool = ctx.enter_context(tc.tile_pool(name="gtp", bufs=8 if GATHER_FP8 else 4))
    hidt_pool = ctx.enter_context(tc.tile_pool(name="hidtp", bufs=2))
    exp_pool = ctx.enter_context(tc.tile_pool(name="expp", bufs=2))
    sm_pool = ctx.enter_context(tc.tile_pool(name="smp", bufs=6))

    if PSUM_PLAN == "Y":
        shared = ctx.enter_context(tc.tile_pool(name="psshared", bufs=4, space="PSUM"))
        ps_means = ps_cnt = ps_pre = ps_logits = ps_expt = shared
        ps_pstack = ctx.enter_context(tc.tile_pool(name="pspstack", bufs=2, space="PSUM"))
        _ret = True
    else:
        _ret = False
    if _ret:
        pass
    elif PSUM_PLAN == "W":
        # all single-bank tiles share one ring; pstack double-buffers
        # (measured 217us vs X's 162us back-to-back: X wins)
        shared = ctx.enter_context(tc.tile_pool(name="psshared", bufs=2, space="PSUM"))
        ps_means = ps_cnt = ps_pre = ps_logits = ps_expt = shared
        ps_pstack = ctx.enter_context(tc.tile_pool(name="pspstack", bufs=2, space="PSUM"))
    elif PSUM_PLAN == "X":
        # pre/lg tiles are [P, 1024] = 2 banks each: 2x2 + pstack 2 + cnt 1
        # + means 1 = 8 banks
        shared = ctx.enter_context(tc.tile_pool(name="psshared", bufs=2, space="PSUM"))
        ps_pre = ps_logits = ps_expt = shared
        ps_means = ctx.enter_context(tc.tile_pool(name="psmeans", bufs=1, space="PSUM"))
        ps_cnt = ctx.enter_context(tc.tile_pool(name="pscnt", bufs=1, space="PSUM"))
        ps_pstack = ctx.enter_context(tc.tile_pool(name="pspstack", bufs=1, space="PSUM"))
    else:
        ps_means = ctx.enter_context(tc.tile_pool(name="psmeans", bufs=1, space="PSUM"))
        ps_cnt = ctx.enter_context(tc.tile_pool(name="pscnt", bufs=1, space="PSUM"))
        ps_pstack = ctx.enter_context(tc.tile_pool(name="pspstack", bufs=PSTACK_BUFS, space="PSUM"))
        ps_pre = ctx.enter_context(tc.tile_pool(name="pspre", bufs=PRE_BUFS, space="PSUM"))
        ps_logits = ctx.enter_context(tc.tile_pool(name="pslogits", bufs=1, space="PSUM"))
        ps_expt = ctx.enter_context(tc.tile_pool(name="psexpt", bufs=1, space="PSUM"))

    def _head(b):
        """Input DMAs + casts + A one-hot for sentence b. Emitted one
        sentence ahead (mid-way through the previous sentence) so the
        DMA/DVE prefetch chain is prioritized before the previous
        sentence's tail work."""
        seg_i = seg_pool.tile([P, SCH], i32, tag="segi", name="seg_i")
        nc.sync.dma_start(seg_i[:], seg_d[b].rearrange("(q p) -> p q", p=P))
        seg_row = None
        if CNT_VIA_DVE:
            seg_row = seg_pool.tile([1, S], i32, tag="segrow", name="seg_row", bufs=2)
            nc.sync.dma_start(seg_row[:], seg_d[b].unsqueeze(0))
        conf_row = conf_pool.tile([1, 2, C], i32, name="confrow", tag="confrow")
        nc.sync.dma_start(conf_row[:], conf_d[b].rearrange("c h -> h c").unsqueeze(0))
        conf_rows = [conf_row[:, 0, :], conf_row[:, 1, :]]

        # one DMA for the whole sentence's h (q-chunks along a free dim)
        hfull = hf_pool.tile([P, SCH, D], f32r if MEANS_F32R else f32,
                             name="hf", tag="hf", bufs=2)
        h_src = h_d[b * S:(b + 1) * S, :].rearrange("(q p) d -> p q d", p=P)
        if MEANS_F32R:
            h_src = h_src.bitcast(f32r)
        nc.sync.dma_start(hfull[:], h_src)
        if MEANS_F32R:
            hb = None
        else:
            hb = []
            for q in range(SCH):
                t = hb_pool.tile([P, D], bf16, name=f"hb{q}", tag="hb")
                for ch in range(2):
                    _cast(nc, t[:, ch * (D // 2):(ch + 1) * (D // 2)],
                          hfull[:, q, ch * (D // 2):(ch + 1) * (D // 2)],
                          HB_CAST_ENGINES[q])
                hb.append(t)

        seg_f = seg_pool.tile([P, SCH], f32, tag="segf", name="seg_f")
        _cast(nc, seg_f[:], seg_i[:])

        A = []
        for q in range(SCH):
            if MEANS_F32R:
                t = a_pool.tile([P, T], f32r, name=f"A{q}", tag="A")
                nc.vector.tensor_scalar(t[:], iota_row[:], seg_f[:, q:q + 1], 0.0,
                                        Alu.subtract, Alu.is_equal)
            else:
                # bf16 in/out -> DVE 2x mode
                t = a_pool.tile([P, T], bf16, name=f"A{q}", tag="A")
                nc.vector.tensor_scalar(t[:], iota_row_bf[:], seg_f[:, q:q + 1], 0.0,
                                        Alu.subtract, Alu.is_equal)
            A.append(t)

        # counts + reciprocals (depend only on seg: prefetch a sentence ahead)
        recips = []
        if CNT_VIA_DVE:
            segbc = idxbc_pool.tile([P, S], i32, tag="segbc", name="segbc", bufs=2)
            nc.gpsimd.partition_broadcast(segbc[:], seg_row[:])
            for mt in range(TCH):
                scr = a_pool.tile([P, S], bf16, tag="scr", name="scr", bufs=2)
                craw = recip_pool.tile([P, 1], f32, name="craw", tag="craw", bufs=4)
                nc.vector.tensor_scalar(scr[:], segbc[:],
                                        iota_col[:] if mt == 0 else iota_col2[:],
                                        None, Alu.is_equal)
                nc.vector.tensor_reduce(craw[:], scr[:], mybir.AxisListType.X,
                                        Alu.add)
                csb = recip_pool.tile([P, 1], f32, name="cnt", tag="cnt", bufs=4)
                nc.vector.tensor_scalar(csb[:], craw[:], 1.0, None, Alu.max)
                r = recip_pool.tile([P, 1], f32, name="recip", tag="recip", bufs=8)
                nc.vector.reciprocal(r[:], csb[:])
                recips.append(r)

        # G^T one-hot [512, 1024]: G^T[t, c] = (conf[c, half] - 1 == t),
        # q-pair tiles [t, j, c] for the DoubleRow gather. Depends only on
        # conf, so prefetched here a sentence ahead of its gather matmuls.
        idx_bc = []
        for half in range(2):
            t = idxbc_pool.tile([P, C], i32, name=f"idxbc{half}", tag="idxbc")
            nc.gpsimd.partition_broadcast(t[:], conf_rows[half][:])
            idx_bc.append(t)
        gdt_h = fp8 if GATHER_FP8 else bf16
        GT = [gt_pool.tile([P, 2, C], gdt_h, name=f"GT{j}", tag="GT")
              for j in range(SCH // 2)]
        for q in range(SCH):
            tbase = P * (q % TCH)
            nc.vector.tensor_scalar(GT[q // 2][:, q % 2, :], idx_bc[q // TCH][:],
                                    iota_col[:], float(tbase + 1),
                                    Alu.subtract, Alu.is_equal)
        return dict(conf_rows=conf_rows, hb=hb, hfull=hfull, A=A, seg_row=seg_row,
                    GT=GT, recips=recips)

    blist = [bb for _ in range(REPEATS) for bb in range(BPC)]
    heads = {0: _head(blist[0])}
    for bi, b in enumerate(blist):
        if bi not in heads:
            heads[bi] = _head(blist[bi])
        st_h = heads.pop(bi)
        conf_rows, hb, A = st_h["conf_rows"], st_h["hb"], st_h["A"]
        seg_row, hfull = st_h["seg_row"], st_h["hfull"]

        # ---- means^T [768, 256] = h^T @ A (6 m-chunks; d-pair tiles for
        # the DoubleRow P_stack matmul) ----
        mdt = fp8 if PSTACK_FP8 else bf16
        means = [means_pool.tile([P, 2, T], mdt, name=f"means{kk}", tag="means")
                 for kk in range(DCH // 2)]
        for kk in range(DCH // 2):
            # two m-chunks share one 1-bank psum tile -> single wide eviction
            mps = ps_means.tile([P, 512], f32, name="mps", tag="shared" if PSUM_PLAN in ("X", "Y") and ps_means is ps_cnt else "")
            for i in range(2):
                m = 2 * kk + i
                for q in range(SCH):
                    lhs = (hfull[:, q, m * P:(m + 1) * P] if MEANS_F32R
                           else hb[q][:, m * P:(m + 1) * P])
                    nc.tensor.matmul(mps[:, i * T:(i + 1) * T], lhs, A[q][:],
                                     start=(q == 0), stop=(q == SCH - 1))
            _cast(nc, means[kk][:, :, :], mps[:], MEANS_EVICT_ENGINE)

        # ---- counts + reciprocal (per token, column layout) ----
        if CNT_VIA_DVE:
            recips = st_h["recips"]  # prefetched in the head
        else:
            recips = []
            for mt in range(TCH):
                cps = ps_cnt.tile([P, 512], f32, name="cps", tag="shared" if PSUM_PLAN in ("X", "Y") else "")[:, 0:1]
                for q in range(SCH):
                    nc.tensor.matmul(cps[:], A[q][:, mt * P:(mt + 1) * P],
                                     ones_f[:],
                                     start=(q == 0), stop=(q == SCH - 1))
                csb = recip_pool.tile([P, 1], f32, name="cnt", tag="cnt", bufs=4)
                nc.vector.tensor_scalar(csb[:], cps[:], 1.0, None, Alu.max)
                r = recip_pool.tile([P, 1], f32, name="recip", tag="recip", bufs=8)
                nc.vector.reciprocal(r[:], csb[:])
                recips.append(r)

        if PIPELINE_HEAD and HEAD_AFTER == "cnt" and bi + 1 < len(blist):
            heads[bi + 1] = _head(blist[bi + 1])
        if not wt:
            _load_weights()
        w1s, w2s, b1s, b2c = wt["w1s"], wt["w2s"], wt["b1s"], wt["b2c"]

        GT = st_h["GT"]
        gdt = fp8 if GATHER_FP8 else bf16

        # ---- P_stack [512, 640] = means @ W1half, scaled by 1/cnt ----
        pstack = [pstack_pool.tile([P, 2, MLPP], gdt, name=f"pstack{j}", tag="pstack")
                  for j in range(SCH // 2)]
        for mq in range(SCH):
            half = mq // TCH
            tq = mq % TCH
            pps = ps_pstack.tile([P, MLPP], f32)
            for ns, ne in ((0, 512), (512, MLPP)):
                if PSTACK_FP8:
                    for kk in range(DCH // 2):
                        nc.tensor.matmul(pps[:, ns:ne],
                                         means[kk][:, :, tq * P:(tq + 1) * P],
                                         w1s[:, kk, :, half * MLPP + ns: half * MLPP + ne],
                                         start=(kk == 0), stop=(kk == DCH // 2 - 1),
                                         perf_mode=DR)
                else:
                    for k in range(DCH):
                        nc.tensor.matmul(pps[:, ns:ne],
                                         means[k // 2][:, k % 2, tq * P:(tq + 1) * P],
                                         w1s[:, k // 2, k % 2, half * MLPP + ns: half * MLPP + ne],
                                         start=(k == 0), stop=(k == DCH - 1))
            pdst = pstack[mq // 2][:, mq % 2, :]
            if PSTACK_EVICT_ENGINE == "scalar":
                nc.scalar.activation(pdst, pps[:], Act.Copy, scale=recips[tq][:])
            elif PSTACK_EVICT_ENGINE == "gpsimd":
                nc.gpsimd.tensor_scalar(pdst, pps[:], recips[tq][:], None, Alu.mult)
            else:
                nc.vector.tensor_scalar(pdst, pps[:], recips[tq][:], None, Alu.mult)

        if PIPELINE_HEAD and HEAD_AFTER == "pstack" and bi + 1 < len(blist):
            heads[bi + 1] = _head(blist[bi + 1])

        # ---- pre^T = P_stack^T-gather, tanh -> hid^T [640, 1024] ----
        hdt = fp8 if LOGITS_FP8 else bf16
        hidT = hidt_pool.tile([P, MCH, C], hdt)
        for m in range(MCH):
            # both n2 halves into one 2-bank psum tile -> single wide tanh
            pre = ps_pre.tile([P, C], f32, name="pre", tag="shared" if PSUM_PLAN in ("X", "Y") else "")
            for n2 in range(NH):
                if GATHER_FP8:
                    for j in range(SCH // 2):
                        nc.tensor.matmul(pre[:, n2 * 512:(n2 + 1) * 512],
                                         pstack[j][:, :, m * P:(m + 1) * P],
                                         GT[j][:, :, n2 * 512:(n2 + 1) * 512],
                                         start=(j == 0), stop=(j == SCH // 2 - 1),
                                         perf_mode=DR)
                else:
                    for q in range(SCH):
                        nc.tensor.matmul(pre[:, n2 * 512:(n2 + 1) * 512],
                                         pstack[q // 2][:, q % 2, m * P:(m + 1) * P],
                                         GT[q // 2][:, q % 2, n2 * 512:(n2 + 1) * 512],
                                         start=(q == 0), stop=(q == SCH - 1))
            nc.scalar.activation(hidT[:, m, :], pre[:],
                                 Act.Tanh, bias=b1s[:, m:m + 1])

        # ---- logits^T [4, 1024] = W2^T @ hid^T; exp(+b2) ----
        smdt = bf16 if SOFTMAX_BF16 else f32
        exp_sb = exp_pool.tile([32 if SOFTMAX_ENGINE == "dve" else CLS, C], smdt)
        if SOFTMAX_ENGINE == "dve":
            # define rows 4-31 (transposed into lanes k>=4, never read
            # downstream) so the block transpose has owned, initialized input;
            # runs on the mostly-idle gpsimd, ACT then overwrites rows 0-3
            nc.gpsimd.memset(exp_sb[:, :], 0.0)
        lg = ps_logits.tile([P, C], f32, name="lg", tag="shared" if PSUM_PLAN in ("X", "Y") else "")
        for n2 in range(NH):
            for m in range(MCH):
                nc.tensor.matmul(lg[:CLS, n2 * 512:(n2 + 1) * 512], w2s[:, m, :],
                                 hidT[:, m, n2 * 512:(n2 + 1) * 512],
                                 start=(m == 0), stop=(m == MCH - 1))
        nc.scalar.activation(exp_sb[:CLS, :], lg[:CLS, :], Act.Exp, bias=b2c[:])

        # ---- transpose exp^T -> config-partition layout; normalize; DMA out ----
        if SOFTMAX_ENGINE == "dve":
            # DVE 32x32-block transpose: out[p', 32j+k] = exp[k, 32j+p'];
            # config c = 32j + p', classes at k<4 (k>=4 is junk, never read).
            et = sm_pool.tile([32, C], smdt, name="et", tag="et", bufs=2)
            nc.vector.transpose(et[:], exp_sb[:])
            etv = et[:].rearrange("p (j k) -> p j k", k=32)[:, :, 0:CLS]
            den = sm_pool.tile([32, 32], f32, name="den", tag="den", bufs=2)
            nc.vector.tensor_reduce(den[:], etv, mybir.AxisListType.X, Alu.add)
            rden = sm_pool.tile([32, 32], f32, name="rden", tag="rden", bufs=2)
            nc.vector.reciprocal(rden[:], den[:])
            sm = sm_pool.tile([32, 32, CLS], f32, name="sm", tag="sm", bufs=2)
            nc.vector.tensor_tensor(sm[:], etv,
                                    rden[:].unsqueeze(-1).broadcast_to((32, 32, CLS)),
                                    Alu.mult)
            nc.sync.dma_start(
                out_d[b * C:(b + 1) * C, :].rearrange("(j p) k -> p j k", p=32),
                sm[:])
        else:
            expT = ps_expt.tile([P, 512], f32, name="expT", tag="shared" if PSUM_PLAN in ("X", "Y") else "")[:, 0:CJ * CLS]
            for j in range(CJ):
                nc.tensor.transpose(expT[:, j * CLS:(j + 1) * CLS],
                                    exp_sb[:, j * P:(j + 1) * P], ident4[:])
            den = sm_pool.tile([P, CJ], f32, name="den", tag="den", bufs=2)
            nc.vector.tensor_reduce(den[:], expT[:].rearrange("p (j k) -> p j k", k=CLS),
                                    mybir.AxisListType.X, Alu.add)
            rden = sm_pool.tile([P, CJ], f32, name="rden", tag="rden", bufs=2)
            nc.vector.reciprocal(rden[:], den[:])
            sm = sm_pool.tile([P, CJ * CLS], f32, name="sm", tag="sm", bufs=2)
            nc.vector.tensor_tensor(sm[:].rearrange("p (j k) -> p j k", k=CLS),
                                    expT[:].rearrange("p (j k) -> p j k", k=CLS),
                                    rden[:].unsqueeze(-1).broadcast_to((P, CJ, CLS)),
                                    Alu.mult)
            nc.sync.dma_start(
                out_d[b * C:(b + 1) * C, :].rearrange("(j p) k -> p j k", p=P),
                sm[:].rearrange("p (j k) -> p j k", k=CLS))


def build_module():
    nc = bass.Bass("TRN2", target_bir_lowering=False, debug=False)

    h_d = nc.dram_tensor("h", [BPC * S, D], f32, kind="ExternalInput").ap()
    seg_d = nc.dram_tensor("seg", [BPC, S], i32, kind="ExternalInput").ap()
    conf_d = nc.dram_tensor("conf", [BPC, C, 2], i32, kind="ExternalInput").ap()
    w1_d = nc.dram_tensor("w1", [2 * D, MLP], f32, kind="ExternalInput").ap()
    b1_d = nc.dram_tensor("b1", [MLP], f32, kind="ExternalInput").ap()
    w2_d = nc.dram_tensor("w2", [MLP, CLS], f32, kind="ExternalInput").ap()
    b2_d = nc.dram_tensor("b2", [CLS], f32, kind="ExternalInput").ap()
    out_d = nc.dram_tensor("out", [BPC * C, CLS], f32, kind="ExternalOutput").ap()

    iota_row_d = nc.inline_tensor(
        np.broadcast_to(np.arange(T, dtype=np.float32), (P, T)).copy(), "c_iota_row").ap()
    iota_col_d = nc.inline_tensor(
        np.arange(P, dtype=np.float32).reshape(P, 1), "c_iota_col").ap()
    ident_d = nc.inline_tensor(np.eye(CLS, dtype=np.float32), "c_ident").ap()

    with tile.TileContext(nc) as tc:
        with ExitStack() as ctx:
            nc.gpsimd.load_library(library_config.mlp)
            _body(ctx, tc, nc, h_d, seg_d, conf_d, w1_d, b1_d, w2_d, b2_d, out_d,
                  iota_row_d, iota_col_d, ident_d)
    # Raw Bass skips several Bacc.compile() passes the NEFF compiler needs:
    # - move_matmul_waits_to_ldweights + generate_event_semaphores: TRN2 allows
    #   at most 1 sync wait per instruction ("Too many sync wait commands")
    # - codegen_inst_isa_subclasses: fills .instr bytes for the gpsimd
    #   extended-ISA ops ("ISA wrong length")
    import bass_rust as _bass_rust
    _bass_rust.move_matmul_waits_to_ldweights(nc.m)
    _bass_rust.generate_event_semaphores(nc)
    mybir.codegen_inst_isa_subclasses(nc)
    return nc


_NC = None


def _get_nc():
    global _NC
    if _NC is None:
        _NC = build_module()
    return _NC


_RUNNER = None


def _get_runner():
    """Build the jitted PJRT callable once per process (run_bass_kernel_spmd
    retraces jax on every call, which costs seconds)."""
    global _RUNNER
    if _RUNNER is not None:
        return _RUNNER
    import jax
    from jax.sharding import Mesh, PartitionSpec
    from jax.experimental.shard_map import shard_map
    from concourse import bass2jax

    nc = _get_nc()
    bass2jax.install_neuronx_cc_hook()
    partition_name = nc.partition_id_tensor.name if nc.partition_id_tensor else None
    in_names, out_names, out_avals, out_shapes = [], [], [], []
    for alloc in nc.m.functions[0].allocations:
        if not isinstance(alloc, mybir.MemoryLocationSet):
            continue
        name = alloc.memorylocations[0].name
        if alloc.kind == "ExternalInput":
            if name != partition_name:
                in_names.append(name)
        elif alloc.kind == "ExternalOutput":
            shape = tuple(alloc.tensor_shape)
            dtype = mybir.dt.np(alloc.dtype)
            out_avals.append(jax.core.ShapedArray(shape, dtype))
            out_names.append(name)
            out_shapes.append((shape, dtype))
    all_in_names = list(in_names) + list(out_names)
    if partition_name is not None:
        all_in_names.append(partition_name)

    def _pjrt_body(*args):
        operands = list(args)
        if partition_name is not None:
            operands.append(bass2jax.partition_id_tensor())
        return tuple(bass2jax._bass_exec_p.bind(
            *operands,
            out_avals=tuple(out_avals),
            in_names=tuple(all_in_names),
            out_names=tuple(out_names),
            lowering_input_output_aliases=(),
            sim_require_finite=True,
            sim_require_nnan=True,
            nc=nc,
        ))

    devices = jax.devices()[:NCORES]
    mesh = Mesh(np.asarray(devices), ("core",))
    n_outs = len(out_names)
    in_specs = (PartitionSpec("core"),) * (len(in_names) + n_outs)
    out_specs = (PartitionSpec("core"),) * n_outs
    fn = jax.jit(shard_map(_pjrt_body, mesh=mesh, in_specs=in_specs,
                           out_specs=out_specs, check_rep=False),
                 keep_unused=True)
    _RUNNER = (fn, in_names, out_names, out_shapes)
    return _RUNNER


def run_cached(in_maps):
    """Execute via the cached jit; returns list of per-core {name: np.ndarray}."""
    fn, in_names, out_names, out_shapes = _get_runner()
    concat_in = [np.concatenate([in_maps[c][n] for c in range(NCORES)], axis=0)
                 for n in in_names]
    concat_zeros = [np.zeros((NCORES * s[0], *s[1:]), dt)
                    for (s, dt) in out_shapes]
    out_arrs = fn(*concat_in, *concat_zeros)
    res = []
    for c in range(NCORES):
        res.append({name: np.asarray(out_arrs[i]).reshape(
            NCORES, *out_shapes[i][0])[c] for i, name in enumerate(out_names)})
    return res


def make_in_maps(h, seg_ids, conf, W1, b1, W2, b2):
    h = np.ascontiguousarray(np.asarray(h), dtype=np.float32)
    seg_ids = np.ascontiguousarray(np.asarray(seg_ids), dtype=np.int32)
    conf = np.ascontiguousarray(np.asarray(conf), dtype=np.int32)
    W1 = np.ascontiguousarray(np.asarray(W1), dtype=np.float32)
    b1 = np.ascontiguousarray(np.asarray(b1), dtype=np.float32)
    W2 = np.ascontiguousarray(np.asarray(W2), dtype=np.float32)
    b2 = np.ascontiguousarray(np.asarray(b2), dtype=np.float32)
    in_maps = []
    for i in range(NCORES):
        sl = slice(i * BPC, (i + 1) * BPC)
        in_maps.append({
            "h": h[sl].reshape(BPC * S, D),
            "seg": seg_ids[sl],
            "conf": conf[sl],
            "w1": W1, "b1": b1, "w2": W2, "b2": b2,
        })
    return in_maps


def run(in_maps, trace=False, **kwargs):
    nc = _get_nc()
    return run_bass_kernel_spmd(nc, in_maps, core_ids=list(range(NCORES)),
                                trace=trace, **kwargs)


def kernel(h, seg_ids, conf, W1, b1, W2, b2):
    global _RUNNER
    in_maps = make_in_maps(h, seg_ids, conf, W1, b1, W2, b2)
    # The axon-tunneled devices occasionally fail the first execution after a
    # fresh NEFF load (NRT_EXEC_UNIT_UNRECOVERABLE); a retry on a rebuilt
    # executable has always succeeded. Guard the graded call.
    last = None
    for attempt in range(3):
        try:
            res = run_cached(in_maps)
            break
        except Exception as e:  # noqa: BLE001
            last = e
            _RUNNER = None
            import time as _time
            _time.sleep(2.0 * (attempt + 1))
    else:
        raise last
    outs = [res[i]["out"] for i in range(NCORES)]
    return np.concatenate(outs, axis=0)



# revision 76
# speedup vs baseline: 1.0206x; 1.0206x over previous
"""Trainium2 Bass kernel for nn_BERTNet_75256416961146.

Pipeline per sentence (B=64 sentences, sharded 8/core over 8 NeuronCores):
  1. segment-mean of h[b] [512,768] over sorted seg_ids -> means [256,768]
     (computed transposed as means^T [768,256] = h^T @ A, A = one-hot of seg_ids)
  2. P_stack [512,640] = [means @ W1_top ; means @ W1_bot] (row r is the
     left/right MLP projection of token r mod 256), scaled by 1/cnt per token,
     evicted to fp8e4 q-pair tiles
  3. pre^T [640,1024] = P_stack^T-gather via fp8e4 DoubleRow matmuls against
     the one-hot G^T (exact in fp8); tanh(+b1) in one [128,1024] ACT op per m
  4. logits^T [4,1024] = W2^T @ hid^T (bf16); exp(+b2); 8 PE transposes to
     config-partition layout; softmax normalize (tiny DVE ops); DMA out.

means/P_stack/logits matmuls in bf16, gather in fp8e4 DoubleRow (2 k-tiles
per instruction), all with fp32 PSUM accumulation. One DMA per sentence for
h ([p, q, d] layout), single conf DMA, consolidated weight staging.

HW-measured choices (trn2, axon): fp8 DoubleRow gather -40us; bf16 A-build
(2x DVE) large win; f32r matmuls slower than bf16+cast; gpsimd tensor_scalar
and ACT Copy casts catastrophically slow (+100us); bf16 softmax slower.
fp8 on P_stack/logits inputs exceeds the 2e-2 error budget (each stage alone
is ~0.013-0.018 rel; they RSS-combine).
"""

import os
import numpy as np
from contextlib import ExitStack

os.environ.setdefault("MYCRO_LOCAL_CACHE", "1")

import concourse.bass as bass
import concourse.tile as tile
from concourse import mybir
from concourse import library_config
from concourse.bass_utils import run_bass_kernel_spmd

# ---- problem shapes (hardcoded per contest rules) ----
B, S, T, C = 64, 512, 256, 1024
D, MLP, CLS = 768, 600, 4
NCORES = 8
BPC = B // NCORES          # sentences per core
P = 128
SCH = S // P               # 4 subtoken chunks
DCH = D // P               # 6 hidden chunks
TCH = T // P               # 2 token chunks
MLPP = 640                 # MLP padded to 5*128
MCH = MLPP // P            # 5 mlp chunks
NH = C // 512              # 2 config halves for psum tiling
CJ = C // P                # 8 config blocks of 128

f32 = mybir.dt.float32
bf16 = mybir.dt.bfloat16
i32 = mybir.dt.int32
Alu = mybir.AluOpType
Act = mybir.ActivationFunctionType

# Timing aid: build the module with the whole per-sentence body repeated
# REPEATS times (same data, same outputs) so kernel time can be separated
# from host dispatch overhead. Grading path always uses REPEATS=1.
REPEATS = 1

# ---- tuning knobs (swept via TimelineSim, validated on HW) ----
def _knob(name, default):
    return os.environ.get(f"K_{name}", default)


def _flag(name, default):
    return os.environ.get(f"K_{name}", "1" if default else "0") == "1"


MEANS_EVICT_ENGINE = _knob("MEANS_EVICT", "vector")  # PSUM->SBUF means eviction
PSTACK_EVICT_ENGINE = _knob("PSTACK_EVICT", "vector")  # scale+cast eviction engine
# h f32->bf16 cast engines, one per q-chunk (when MEANS_F32R off)
HB_CAST_ENGINES = _knob("HB_CAST", "vector,vector,vector,vector").split(",")
PSTACK_BUFS = int(_knob("PSTACK_BUFS", "1"))  # psum bufs for P_stack tiles (2 banks each)
PRE_BUFS = int(_knob("PRE_BUFS", "2"))        # psum bufs for pre tiles (1 bank each)
PSUM_PLAN = _knob("PSUM_PLAN", "X")  # "A": separate pools; "X": cnt/pre/logits/expT share one
PIPELINE_HEAD = True         # emit next sentence's loads/casts/A mid-way through current
HEAD_AFTER = _knob("HEAD_AFTER", "cnt")  # emit next head after cnt (155 vs 168us A/B)
CNT_IN_HEAD = _flag("CNT_IN_HEAD", False)  # cnt prefetch in head measured 150 vs 140us: off
# counts on DVE (compare+reduce, needs segbc broadcast) vs 8 tiny bf16 PE
# matmuls; PE variant measured slower back-to-back (208 vs 190us)
CNT_VIA_DVE = _flag("CNT_VIA_DVE", True)
SOFTMAX_ENGINE = _knob("SOFTMAX_ENGINE", "pe")  # "pe": 8 PE transposes; "dve": DVE 32x32 transpose
SOFTMAX_BF16 = _flag("SOFTMAX_BF16", False)  # bf16 softmax measured slower on HW
MEANS_F32R = _flag("MEANS_F32R", False)  # f32r matmuls measured slower than bf16+cast on HW
PSTACK_FP8 = _flag("PSTACK_FP8", False)  # fp8 means+W1 alone costs rel-err 0.024: off
GATHER_FP8 = _flag("GATHER_FP8", True)   # pre-gather matmul in fp8e4 DoubleRow (~0.014 err)
LOGITS_FP8 = _flag("LOGITS_FP8", False)  # no PE win (DoubleRow M=4 invalid) but adds err
f32r = mybir.dt.float32r
fp8 = mybir.dt.float8e4
DR = mybir.MatmulPerfMode.DoubleRow


def _cast(nc, out, in_, engine="vector"):
    # dtype-converting copy. On DVE/gpsimd use tensor_scalar (lowers to
    # TensorScalarPtr, which supports multiple sync waits; DVE TensorCopy's
    # TR struct does not). On the scalar engine use an Activation copy.
    if engine == "scalar":
        nc.scalar.activation(out, in_, Act.Copy)
    elif engine == "gpsimd":
        nc.gpsimd.tensor_scalar(out, in_, 1.0, None, Alu.mult)
    else:
        nc.vector.tensor_scalar(out, in_, 1.0, None, Alu.mult)


def _body(ctx, tc, nc, h_d, seg_d, conf_d, w1_d, b1_d, w2_d, b2_d, out_d,
          iota_row_d, iota_col_d, ident_d):
    const = ctx.enter_context(tc.tile_pool(name="const", bufs=1))
    wstage = ctx.enter_context(tc.tile_pool(name="wstage", bufs=1))

    # ---------- constants ----------
    iota_row = const.tile([P, T], f32)          # 0..255 along free, all partitions
    nc.sync.dma_start(iota_row[:], iota_row_d)
    iota_row_bf = const.tile([P, T], bf16)      # bf16 copy: A build runs 2x on DVE
    _cast(nc, iota_row_bf[:], iota_row[:])
    iota_col = const.tile([P, 1], f32)          # partition index
    nc.sync.dma_start(iota_col[:], iota_col_d)
    iota_col2 = const.tile([P, 1], f32)         # partition index + 128
    nc.vector.tensor_scalar(iota_col2[:], iota_col[:], 128.0, None, Alu.add)
    if SOFTMAX_ENGINE == "pe":
        ident4 = const.tile([CLS, CLS], f32)
        nc.sync.dma_start(ident4[:], ident_d)
    ones_f = const.tile([P, 1], f32r if MEANS_F32R else bf16)
    # memset can't write f32r; synthesize 1.0 = iota*0 + 1 on DVE instead
    nc.vector.tensor_scalar(ones_f[:], iota_col[:], 0.0, 1.0, Alu.mult, Alu.add)

    # ---------- weights (emitted lazily, after sentence-0's input DMAs, so
    # the first means matmuls are not starved behind 3.6MB of W1 DMA) ----------
    wt = {}

    def _load_weights():
        # W1 staged as [d-pair, j, half*MLPP + m] for DoubleRow k-tiles (j in
        # {0,1} selects the d-chunk within the pair). One DMA for all of W1.
        w1dt = fp8 if PSTACK_FP8 else bf16
        w1s = const.tile([P, DCH // 2, 2, 2 * MLPP], w1dt)
        for half in range(2):  # zero only the MLP..MLPP pad columns
            nc.vector.memset(w1s[:, :, :, half * MLPP + MLP: (half + 1) * MLPP], 0.0)
        for half in range(2):
            st = wstage.tile([P, DCH, MLP], f32, name="st", tag="st")
            nc.sync.dma_start(st[:], w1_d[half * D:(half + 1) * D, :].rearrange(
                "(g p) m -> p g m", p=P))
            for k in range(DCH):
                _cast(nc, w1s[:, k // 2, k % 2, half * MLPP: half * MLPP + MLP],
                      st[:, k, :])

        w2dt = fp8 if LOGITS_FP8 else bf16
        w2s = const.tile([P, MCH, CLS], w2dt)
        nc.vector.memset(w2s[:, MCH - 1, :], 0.0)
        st2 = wstage.tile([P, 4, CLS], f32, name="st2", tag="st2")
        nc.sync.dma_start(st2[:], w2_d[0:512, :].rearrange("(m p) c -> p m c", p=P))
        st2b = wstage.tile([P, CLS], f32, name="st2b", tag="st2b")
        nc.sync.dma_start(st2b[:MLP - 512], w2_d[512:MLP, :])
        for m in range(4):
            _cast(nc, w2s[:, m, :], st2[:, m, :])
        _cast(nc, w2s[:MLP - 512, MCH - 1, :], st2b[:MLP - 512])

        b1s = const.tile([P, MCH], f32)
        nc.vector.memset(b1s[:], 0.0)
        nc.sync.dma_start(b1s[:, 0:4], b1_d[0:512].rearrange("(m p) -> p m", p=P))
        nc.sync.dma_start(b1s[:MLP - 512, 4:5], b1_d[512:MLP].unsqueeze(-1))

        b2c = const.tile([CLS, 1], f32)
        nc.sync.dma_start(b2c[:], b2_d.unsqueeze(-1))
        wt.update(w1s=w1s, w2s=w2s, b1s=b1s, b2c=b2c)

    # ---------- per-sentence pools ----------
    hf_pool = ctx.enter_context(tc.tile_pool(name="hf", bufs=3))
    hb_pool = None if MEANS_F32R else ctx.enter_context(tc.tile_pool(name="hb", bufs=12))
    seg_pool = ctx.enter_context(tc.tile_pool(name="segp", bufs=6))
    a_pool = ctx.enter_context(tc.tile_pool(name="ap", bufs=8))
    means_pool = ctx.enter_context(tc.tile_pool(name="meansp", bufs=8))
    recip_pool = ctx.enter_context(tc.tile_pool(name="recipp", bufs=8))
    pstack_pool = ctx.enter_context(tc.tile_pool(name="pstackp", bufs=8 if GATHER_FP8 else 4))
    conf_pool = ctx.enter_context(tc.tile_pool(name="confp", bufs=2))
    idxbc_pool = ctx.enter_context(tc.tile_pool(name="idxbcp", bufs=2))
    gt_pool = ctx.enter_context(tc.tile_pool(name="gtp", bufs=8 if GATHER_FP8 else 4))
    hidt_pool = ctx.enter_context(tc.tile_pool(name="hidtp", bufs=2))
    exp_pool = ctx.enter_context(tc.tile_pool(name="expp", bufs=2))
    sm_pool = ctx.enter_context(tc.tile_pool(name="smp", bufs=6))

    if PSUM_PLAN == "Y":
        shared = ctx.enter_context(tc.tile_pool(name="psshared", bufs=4, space="PSUM"))
        ps_means = ps_cnt = ps_pre = ps_logits = ps_expt = shared
        ps_pstack = ctx.enter_context(tc.tile_pool(name="pspstack", bufs=2, space="PSUM"))
        _ret = True
    else:
        _ret = False
    if _ret:
        pass
    elif PSUM_PLAN == "W":
        # all single-bank tiles share one ring; pstack double-buffers
        # (measured 217us vs X's 162us back-to-back: X wins)
        shared = ctx.enter_context(tc.tile_pool(name="psshared", bufs=2, space="PSUM"))
        ps_means = ps_cnt = ps_pre = ps_logits = ps_expt = shared
        ps_pstack = ctx.enter_context(tc.tile_pool(name="pspstack", bufs=2, space="PSUM"))
    elif PSUM_PLAN == "X":
        # pre/lg tiles are [P, 1024] = 2 banks each: 2x2 + pstack 2 + cnt 1
        # + means 1 = 8 banks
        shared = ctx.enter_context(tc.tile_pool(name="psshared", bufs=2, space="PSUM"))
        ps_pre = ps_logits = ps_expt = shared
        ps_means = ctx.enter_context(tc.tile_pool(name="psmeans", bufs=1, space="PSUM"))
        ps_cnt = ctx.enter_context(tc.tile_pool(name="pscnt", bufs=1, space="PSUM"))
        ps_pstack = ctx.enter_context(tc.tile_pool(name="pspstack", bufs=1, space="PSUM"))
    else:
        ps_means = ctx.enter_context(tc.tile_pool(name="psmeans", bufs=1, space="PSUM"))
        ps_cnt = ctx.enter_context(tc.tile_pool(name="pscnt", bufs=1, space="PSUM"))
        ps_pstack = ctx.enter_context(tc.tile_pool(name="pspstack", bufs=PSTACK_BUFS, space="PSUM"))
        ps_pre = ctx.enter_context(tc.tile_pool(name="pspre", bufs=PRE_BUFS, space="PSUM"))
        ps_logits = ctx.enter_context(tc.tile_pool(name="pslogits", bufs=1, space="PSUM"))
        ps_expt = ctx.enter_context(tc.tile_pool(name="psexpt", bufs=1, space="PSUM"))

    def _head(b):
        """Input DMAs + casts + A one-hot for sentence b. Emitted one
        sentence ahead (mid-way through the previous sentence) so the
        DMA/DVE prefetch chain is prioritized before the previous
        sentence's tail work."""
        seg_i = seg_pool.tile([P, SCH], i32, tag="segi", name="seg_i")
        nc.sync.dma_start(seg_i[:], seg_d[b].rearrange("(q p) -> p q", p=P))
        seg_row = None
        if CNT_VIA_DVE:
            seg_row = seg_pool.tile([1, S], i32, tag="segrow", name="seg_row", bufs=2)
            nc.sync.dma_start(seg_row[:], seg_d[b].unsqueeze(0))
        conf_row = conf_pool.tile([1, 2, C], i32, name="confrow", tag="confrow")
        nc.sync.dma_start(conf_row[:], conf_d[b].rearrange("c h -> h c").unsqueeze(0))
        conf_rows = [conf_row[:, 0, :], conf_row[:, 1, :]]

        # one DMA for the whole sentence's h (q-chunks along a free dim)
        hfull = hf_pool.tile([P, SCH, D], f32r if MEANS_F32R else f32,
                             name="hf", tag="hf", bufs=2)
        h_src = h_d[b * S:(b + 1) * S, :].rearrange("(q p) d -> p q d", p=P)
        if MEANS_F32R:
            h_src = h_src.bitcast(f32r)
        nc.sync.dma_start(hfull[:], h_src)
        if MEANS_F32R:
            hb = None
        else:
            hb = []
            for q in range(SCH):
                t = hb_pool.tile([P, D], bf16, name=f"hb{q}", tag="hb")
                for ch in range(2):
                    _cast(nc, t[:, ch * (D // 2):(ch + 1) * (D // 2)],
                          hfull[:, q, ch * (D // 2):(ch + 1) * (D // 2)],
                          HB_CAST_ENGINES[q])
                hb.append(t)

        seg_f = seg_pool.tile([P, SCH], f32, tag="segf", name="seg_f")
        _cast(nc, seg_f[:], seg_i[:])

        A = []
        for q in range(SCH):
            if MEANS_F32R:
                t = a_pool.tile([P, T], f32r, name=f"A{q}", tag="A")
                nc.vector.tensor_scalar(t[:], iota_row[:], seg_f[:, q:q + 1], 0.0,
                                        Alu.subtract, Alu.is_equal)
            else:
                # bf16 in/out -> DVE 2x mode
                t = a_pool.tile([P, T], bf16, name=f"A{q}", tag="A")
                nc.vector.tensor_scalar(t[:], iota_row_bf[:], seg_f[:, q:q + 1], 0.0,
                                        Alu.subtract, Alu.is_equal)
            A.append(t)

        # counts + reciprocals (depend only on seg: prefetch a sentence ahead)
        recips = []
        if CNT_VIA_DVE and CNT_IN_HEAD:
            segbc = idxbc_pool.tile([P, S], i32, tag="segbc", name="segbc", bufs=2)
            nc.gpsimd.partition_broadcast(segbc[:], seg_row[:])
            for mt in range(TCH):
                scr = a_pool.tile([P, S], bf16, tag="scr", name="scr", bufs=2)
                craw = recip_pool.tile([P, 1], f32, name="craw", tag="craw", bufs=4)
                nc.vector.tensor_scalar(scr[:], segbc[:],
                                        iota_col[:] if mt == 0 else iota_col2[:],
                                        None, Alu.is_equal)
                nc.vector.tensor_reduce(craw[:], scr[:], mybir.AxisListType.X,
                                        Alu.add)
                csb = recip_pool.tile([P, 1], f32, name="cnt", tag="cnt", bufs=4)
                nc.vector.tensor_scalar(csb[:], craw[:], 1.0, None, Alu.max)
                r = recip_pool.tile([P, 1], f32, name="recip", tag="recip", bufs=8)
                nc.vector.reciprocal(r[:], csb[:])
                recips.append(r)

        # G^T one-hot [512, 1024]: G^T[t, c] = (conf[c, half] - 1 == t),
        # q-pair tiles [t, j, c] for the DoubleRow gather. Depends only on
        # conf, so prefetched here a sentence ahead of its gather matmuls.
        idx_bc = []
        for half in range(2):
            t = idxbc_pool.tile([P, C], i32, name=f"idxbc{half}", tag="idxbc")
            nc.gpsimd.partition_broadcast(t[:], conf_rows[half][:])
            idx_bc.append(t)
        gdt_h = fp8 if GATHER_FP8 else bf16
        GT = [gt_pool.tile([P, 2, C], gdt_h, name=f"GT{j}", tag="GT")
              for j in range(SCH // 2)]
        for q in range(SCH):
            tbase = P * (q % TCH)
            nc.vector.tensor_scalar(GT[q // 2][:, q % 2, :], idx_bc[q // TCH][:],
                                    iota_col[:], float(tbase + 1),
                                    Alu.subtract, Alu.is_equal)
        return dict(conf_rows=conf_rows, hb=hb, hfull=hfull, A=A, seg_row=seg_row,
                    GT=GT, recips=recips)

    blist = [bb for _ in range(REPEATS) for bb in range(BPC)]
    heads = {0: _head(blist[0])}
    for bi, b in enumerate(blist):
        if bi not in heads:
            heads[bi] = _head(blist[bi])
        st_h = heads.pop(bi)
        conf_rows, hb, A = st_h["conf_rows"], st_h["hb"], st_h["A"]
        seg_row, hfull = st_h["seg_row"], st_h["hfull"]

        # ---- means^T [768, 256] = h^T @ A (6 m-chunks; d-pair tiles for
        # the DoubleRow P_stack matmul) ----
        mdt = fp8 if PSTACK_FP8 else bf16
        means = [means_pool.tile([P, 2, T], mdt, name=f"means{kk}", tag="means")
                 for kk in range(DCH // 2)]
        for kk in range(DCH // 2):
            # two m-chunks share one 1-bank psum tile -> single wide eviction
            mps = ps_means.tile([P, 512], f32, name="mps", tag="shared" if PSUM_PLAN in ("X", "Y") and ps_means is ps_cnt else "")
            for i in range(2):
                m = 2 * kk + i
                for q in range(SCH):
                    lhs = (hfull[:, q, m * P:(m + 1) * P] if MEANS_F32R
                           else hb[q][:, m * P:(m + 1) * P])
                    nc.tensor.matmul(mps[:, i * T:(i + 1) * T], lhs, A[q][:],
                                     start=(q == 0), stop=(q == SCH - 1))
            _cast(nc, means[kk][:, :, :], mps[:], MEANS_EVICT_ENGINE)

        # ---- counts + reciprocal (per token, column layout) ----
        if CNT_VIA_DVE and CNT_IN_HEAD:
            recips = st_h["recips"]  # prefetched in the head
        elif CNT_VIA_DVE:
            recips = []
            segbc = idxbc_pool.tile([P, S], i32, tag="segbc", name="segbc", bufs=2)
            nc.gpsimd.partition_broadcast(segbc[:], seg_row[:])
            for mt in range(TCH):
                scr = a_pool.tile([P, S], bf16, tag="scr", name="scr", bufs=2)
                craw = recip_pool.tile([P, 1], f32, name="craw", tag="craw", bufs=4)
                nc.vector.tensor_scalar(scr[:], segbc[:],
                                        iota_col[:] if mt == 0 else iota_col2[:],
                                        None, Alu.is_equal)
                nc.vector.tensor_reduce(craw[:], scr[:], mybir.AxisListType.X,
                                        Alu.add)
                csb = recip_pool.tile([P, 1], f32, name="cnt", tag="cnt", bufs=4)
                nc.vector.tensor_scalar(csb[:], craw[:], 1.0, None, Alu.max)
                r = recip_pool.tile([P, 1], f32, name="recip", tag="recip", bufs=8)
                nc.vector.reciprocal(r[:], csb[:])
                recips.append(r)
        else:
            recips = []
            for mt in range(TCH):
                cps = ps_cnt.tile([P, 512], f32, name="cps", tag="shared" if PSUM_PLAN in ("X", "Y") else "")[:, 0:1]
                for q in range(SCH):
                    nc.tensor.matmul(cps[:], A[q][:, mt * P:(mt + 1) * P],
                                     ones_f[:],
                                     start=(q == 0), stop=(q == SCH - 1))
                csb = recip_pool.tile([P, 1], f32, name="cnt", tag="cnt", bufs=4)
                nc.vector.tensor_scalar(csb[:], cps[:], 1.0, None, Alu.max)
                r = recip_pool.tile([P, 1], f32, name="recip", tag="recip", bufs=8)
                nc.vector.reciprocal(r[:], csb[:])
                recips.append(r)

        if PIPELINE_HEAD and HEAD_AFTER == "cnt" and bi + 1 < len(blist):
            heads[bi + 1] = _head(blist[bi + 1])
        if not wt:
            _load_weights()
        w1s, w2s, b1s, b2c = wt["w1s"], wt["w2s"], wt["b1s"], wt["b2c"]

        GT = st_h["GT"]
        gdt = fp8 if GATHER_FP8 else bf16

        # ---- P_stack [512, 640] = means @ W1half, scaled by 1/cnt ----
        pstack = [pstack_pool.tile([P, 2, MLPP], gdt, name=f"pstack{j}", tag="pstack")
                  for j in range(SCH // 2)]
        for mq in range(SCH):
            half = mq // TCH
            tq = mq % TCH
            pps = ps_pstack.tile([P, MLPP], f32)
            for ns, ne in ((0, 512), (512, MLPP)):
                if PSTACK_FP8:
                    for kk in range(DCH // 2):
                        nc.tensor.matmul(pps[:, ns:ne],
                                         means[kk][:, :, tq * P:(tq + 1) * P],
                                         w1s[:, kk, :, half * MLPP + ns: half * MLPP + ne],
                                         start=(kk == 0), stop=(kk == DCH // 2 - 1),
                                         perf_mode=DR)
                else:
                    for k in range(DCH):
                        nc.tensor.matmul(pps[:, ns:ne],
                                         means[k // 2][:, k % 2, tq * P:(tq + 1) * P],
                                         w1s[:, k // 2, k % 2, half * MLPP + ns: half * MLPP + ne],
                                         start=(k == 0), stop=(k == DCH - 1))
            pdst = pstack[mq // 2][:, mq % 2, :]
            if PSTACK_EVICT_ENGINE == "scalar":
                nc.scalar.activation(pdst, pps[:], Act.Copy, scale=recips[tq][:])
            elif PSTACK_EVICT_ENGINE == "gpsimd":
                nc.gpsimd.tensor_scalar(pdst, pps[:], recips[tq][:], None, Alu.mult)
            else:
                nc.vector.tensor_scalar(pdst, pps[:], recips[tq][:], None, Alu.mult)

        if PIPELINE_HEAD and HEAD_AFTER == "pstack" and bi + 1 < len(blist):
            heads[bi + 1] = _head(blist[bi + 1])

        # ---- pre^T = P_stack^T-gather, tanh -> hid^T [640, 1024] ----
        hdt = fp8 if LOGITS_FP8 else bf16
        hidT = hidt_pool.tile([P, MCH, C], hdt)
        for m in range(MCH):
            # both n2 halves into one 2-bank psum tile -> single wide tanh
            pre = ps_pre.tile([P, C], f32, name="pre", tag="shared" if PSUM_PLAN in ("X", "Y") else "")
            for n2 in range(NH):
                if GATHER_FP8:
                    for j in range(SCH // 2):
                        nc.tensor.matmul(pre[:, n2 * 512:(n2 + 1) * 512],
                                         pstack[j][:, :, m * P:(m + 1) * P],
                                         GT[j][:, :, n2 * 512:(n2 + 1) * 512],
                                         start=(j == 0), stop=(j == SCH // 2 - 1),
                                         perf_mode=DR)
                else:
                    for q in range(SCH):
                        nc.tensor.matmul(pre[:, n2 * 512:(n2 + 1) * 512],
                                         pstack[q // 2][:, q % 2, m * P:(m + 1) * P],
                                         GT[q // 2][:, q % 2, n2 * 512:(n2 + 1) * 512],
                                         start=(q == 0), stop=(q == SCH - 1))
            nc.scalar.activation(hidT[:, m, :], pre[:],
                                 Act.Tanh, bias=b1s[:, m:m + 1])

        # ---- logits^T [4, 1024] = W2^T @ hid^T; exp(+b2) ----
        smdt = bf16 if SOFTMAX_BF16 else f32
        exp_sb = exp_pool.tile([32 if SOFTMAX_ENGINE == "dve" else CLS, C], smdt)
        if SOFTMAX_ENGINE == "dve":
            # define rows 4-31 (transposed into lanes k>=4, never read
            # downstream) so the block transpose has owned, initialized input;
            # runs on the mostly-idle gpsimd, ACT then overwrites rows 0-3
            nc.gpsimd.memset(exp_sb[:, :], 0.0)
        lg = ps_logits.tile([P, C], f32, name="lg", tag="shared" if PSUM_PLAN in ("X", "Y") else "")
        for n2 in range(NH):
            for m in range(MCH):
                nc.tensor.matmul(lg[:CLS, n2 * 512:(n2 + 1) * 512], w2s[:, m, :],
                                 hidT[:, m, n2 * 512:(n2 + 1) * 512],
                                 start=(m == 0), stop=(m == MCH - 1))
        nc.scalar.activation(exp_sb[:CLS, :], lg[:CLS, :], Act.Exp, bias=b2c[:])

        # ---- transpose exp^T -> config-partition layout; normalize; DMA out ----
        if SOFTMAX_ENGINE == "dve":
            # DVE 32x32-block transpose: out[p', 32j+k] = exp[k, 32j+p'];
            # config c = 32j + p', classes at k<4 (k>=4 is junk, never read).
            et = sm_pool.tile([32, C], smdt, name="et", tag="et", bufs=2)
            nc.vector.transpose(et[:], exp_sb[:])
            etv = et[:].rearrange("p (j k) -> p j k", k=32)[:, :, 0:CLS]
            den = sm_pool.tile([32, 32], f32, name="den", tag="den", bufs=2)
            nc.vector.tensor_reduce(den[:], etv, mybir.AxisListType.X, Alu.add)
            rden = sm_pool.tile([32, 32], f32, name="rden", tag="rden", bufs=2)
            nc.vector.reciprocal(rden[:], den[:])
            sm = sm_pool.tile([32, 32, CLS], f32, name="sm", tag="sm", bufs=2)
            nc.vector.tensor_tensor(sm[:], etv,
                                    rden[:].unsqueeze(-1).broadcast_to((32, 32, CLS)),
                                    Alu.mult)
            nc.sync.dma_start(
                out_d[b * C:(b + 1) * C, :].rearrange("(j p) k -> p j k", p=32),
                sm[:])
        else:
            expT = ps_expt.tile([P, 512], f32, name="expT", tag="shared" if PSUM_PLAN in ("X", "Y") else "")[:, 0:CJ * CLS]
            for j in range(CJ):
                nc.tensor.transpose(expT[:, j * CLS:(j + 1) * CLS],
                                    exp_sb[:, j * P:(j + 1) * P], ident4[:])
            den = sm_pool.tile([P, CJ], f32, name="den", tag="den", bufs=2)
            nc.vector.tensor_reduce(den[:], expT[:].rearrange("p (j k) -> p j k", k=CLS),
                                    mybir.AxisListType.X, Alu.add)
            rden = sm_pool.tile([P, CJ], f32, name="rden", tag="rden", bufs=2)
            nc.vector.reciprocal(rden[:], den[:])
            sm = sm_pool.tile([P, CJ * CLS], f32, name="sm", tag="sm", bufs=2)
            nc.vector.tensor_tensor(sm[:].rearrange("p (j k) -> p j k", k=CLS),
                                    expT[:].rearrange("p (j k) -> p j k", k=CLS),
                                    rden[:].unsqueeze(-1).broadcast_to((P, CJ, CLS)),
                                    Alu.mult)
            nc.sync.dma_start(
                out_d[b * C:(b + 1) * C, :].rearrange("(j p) k -> p j k", p=P),
                sm[:].rearrange("p (j k) -> p j k", k=CLS))


def build_module():
    nc = bass.Bass("TRN2", target_bir_lowering=False, debug=False)

    h_d = nc.dram_tensor("h", [BPC * S, D], f32, kind="ExternalInput").ap()
    seg_d = nc.dram_tensor("seg", [BPC, S], i32, kind="ExternalInput").ap()
    conf_d = nc.dram_tensor("conf", [BPC, C, 2], i32, kind="ExternalInput").ap()
    w1_d = nc.dram_tensor("w1", [2 * D, MLP], f32, kind="ExternalInput").ap()
    b1_d = nc.dram_tensor("b1", [MLP], f32, kind="ExternalInput").ap()
    w2_d = nc.dram_tensor("w2", [MLP, CLS], f32, kind="ExternalInput").ap()
    b2_d = nc.dram_tensor("b2", [CLS], f32, kind="ExternalInput").ap()
    out_d = nc.dram_tensor("out", [BPC * C, CLS], f32, kind="ExternalOutput").ap()

    iota_row_d = nc.inline_tensor(
        np.broadcast_to(np.arange(T, dtype=np.float32), (P, T)).copy(), "c_iota_row").ap()
    iota_col_d = nc.inline_tensor(
        np.arange(P, dtype=np.float32).reshape(P, 1), "c_iota_col").ap()
    ident_d = nc.inline_tensor(np.eye(CLS, dtype=np.float32), "c_ident").ap()

    with tile.TileContext(nc) as tc:
        with ExitStack() as ctx:
            nc.gpsimd.load_library(library_config.mlp)
            _body(ctx, tc, nc, h_d, seg_d, conf_d, w1_d, b1_d, w2_d, b2_d, out_d,
                  iota_row_d, iota_col_d, ident_d)
    # Raw Bass skips several Bacc.compile() passes the NEFF compiler needs:
    # - move_matmul_waits_to_ldweights + generate_event_semaphores: TRN2 allows
    #   at most 1 sync wait per instruction ("Too many sync wait commands")
    # - codegen_inst_isa_subclasses: fills .instr bytes for the gpsimd
    #   extended-ISA ops ("ISA wrong length")
    import bass_rust as _bass_rust
    _bass_rust.move_matmul_waits_to_ldweights(nc.m)
    _bass_rust.generate_event_semaphores(nc)
    mybir.codegen_inst_isa_subclasses(nc)
    return nc


_NC = None


def _get_nc():
    global _NC
    if _NC is None:
        _NC = build_module()
    return _NC


_RUNNER = None


def _get_runner():
    """Build the jitted PJRT callable once per process (run_bass_kernel_spmd
    retraces jax on every call, which costs seconds)."""
    global _RUNNER
    if _RUNNER is not None:
        return _RUNNER
    import jax
    from jax.sharding import Mesh, PartitionSpec
    from jax.experimental.shard_map import shard_map
    from concourse import bass2jax

    nc = _get_nc()
    bass2jax.install_neuronx_cc_hook()
    partition_name = nc.partition_id_tensor.name if nc.partition_id_tensor else None
    in_names, out_names, out_avals, out_shapes = [], [], [], []
    for alloc in nc.m.functions[0].allocations:
        if not isinstance(alloc, mybir.MemoryLocationSet):
            continue
        name = alloc.memorylocations[0].name
        if alloc.kind == "ExternalInput":
            if name != partition_name:
                in_names.append(name)
        elif alloc.kind == "ExternalOutput":
            shape = tuple(alloc.tensor_shape)
            dtype = mybir.dt.np(alloc.dtype)
            out_avals.append(jax.core.ShapedArray(shape, dtype))
            out_names.append(name)
            out_shapes.append((shape, dtype))
    all_in_names = list(in_names) + list(out_names)
    if partition_name is not None:
        all_in_names.append(partition_name)

    def _pjrt_body(*args):
        operands = list(args)
        if partition_name is not None:
            operands.append(bass2jax.partition_id_tensor())
        return tuple(bass2jax._bass_exec_p.bind(
            *operands,
            out_avals=tuple(out_avals),
            in_names=tuple(all_in_names),
            out_names=tuple(out_names),
            lowering_input_output_aliases=(),
            sim_require_finite=True,
            sim_require_nnan=True,
            nc=nc,
        ))

    devices = jax.devices()[:NCORES]
    mesh = Mesh(np.asarray(devices), ("core",))
    n_outs = len(out_names)
    in_specs = (PartitionSpec("core"),) * (len(in_names) + n_outs)
    out_specs = (PartitionSpec("core"),) * n_outs
    fn = jax.jit(shard_map(_pjrt_body, mesh=mesh, in_specs=in_specs,
                           out_specs=out_specs, check_rep=False),
                 keep_unused=True)
    _RUNNER = (fn, in_names, out_names, out_shapes)
    return _RUNNER


def run_cached(in_maps):
    """Execute via the cached jit; returns list of per-core {name: np.ndarray}."""
    fn, in_names, out_names, out_shapes = _get_runner()
    concat_in = [np.concatenate([in_maps[c][n] for c in range(NCORES)], axis=0)
                 for n in in_names]
    concat_zeros = [np.zeros((NCORES * s[0], *s[1:]), dt)
                    for (s, dt) in out_shapes]
    out_arrs = fn(*concat_in, *concat_zeros)
    res = []
    for c in range(NCORES):
        res.append({name: np.asarray(out_arrs[i]).reshape(
            NCORES, *out_shapes[i][0])[c] for i, name in enumerate(out_names)})
    return res


def make_in_maps(h, seg_ids, conf, W1, b1, W2, b2):
    h = np.ascontiguousarray(np.asarray(h), dtype=np.float32)
    seg_ids = np.ascontiguousarray(np.asarray(seg_ids), dtype=np.int32)
    conf = np.ascontiguousarray(np.asarray(conf), dtype=np.int32)
    W1 = np.ascontiguousarray(np.asarray(W1), dtype=np.float32)
    b1 = np.ascontiguousarray(np.asarray(b1), dtype=np.float32)
    W2 = np.ascontiguousarray(np.asarray(W2), dtype=np.float32)
    b2 = np.ascontiguousarray(np.asarray(b2), dtype=np.float32)
    in_maps = []
    for i in range(NCORES):
        sl = slice(i * BPC, (i + 1) * BPC)
        in_maps.append({
            "h": h[sl].reshape(BPC * S, D),
            "seg": seg_ids[sl],
            "conf": conf[sl],
            "w1": W1, "b1": b1, "w2": W2, "b2": b2,
        })
    return in_maps


def run(in_maps, trace=False, **kwargs):
    nc = _get_nc()
    return run_bass_kernel_spmd(nc, in_maps, core_ids=list(range(NCORES)),
                                trace=trace, **kwargs)


def kernel(h, seg_ids, conf, W1, b1, W2, b2):
    global _RUNNER
    in_maps = make_in_maps(h, seg_ids, conf, W1, b1, W2, b2)
    # The axon-tunneled devices occasionally fail the first execution after a
    # fresh NEFF load (NRT_EXEC_UNIT_UNRECOVERABLE); a retry on a rebuilt
    # executable has always succeeded. Guard the graded call.
    last = None
    for attempt in range(3):
        try:
            res = run_cached(in_maps)
            break
        except Exception as e:  # noqa: BLE001
            last = e
            _RUNNER = None
            import time as _time
            _time.sleep(2.0 * (attempt + 1))
    else:
        raise last
    outs = [res[i]["out"] for i in range(NCORES)]
    return np.concatenate(outs, axis=0)

